# revision 1
# baseline (speedup 1.0000x reference)
"""Trainium2 Bass kernel for nn_CifarBaselineSNN.

conv1(3->64,3x3,p1) -> BN -> LIF -> avgpool2
conv2(64->128,3x3,p1) -> BN -> LIF -> avgpool2
fc1(8192->256) -> LIF -> fc2(256->10)+b
T=8, B=128. Data-parallel over B across 8 NeuronCores (16 samples/core);
BN statistics are global over the batch -> 2 small AllReduces.

Numerics: convolutions run as bf16 hi/lo weight-split matmuls accumulating in
fp32 PSUM (inputs to conv2 are pooled spikes, exactly representable in bf16;
conv1 inputs are hi/lo split too). LIF state uses the 2^t scaling trick so one
fused DVE op does decay+charge+reset per step.
"""

import sys
import os

for _p in ("/opt/trn_rl_repo", "/root/.axon_site/_ro/trn_rl_repo"):
    if os.path.isdir(_p) and _p not in sys.path:
        sys.path.append(_p)

import numpy as np

import concourse.bass as bass
import concourse.mybir as mybir
import concourse.tile as tile
from concourse import bacc
from concourse import bass_utils
from concourse import dve_ops as _dops
from concourse.dve_uop import DveOpSpec
from concourse.dve_spec import (
    Spec, Src0, Src1, C0, C1, C2, Zero, AluOp, sq, select, lower, _has_src1,
)

F32 = mybir.dt.float32
BF16 = mybir.dt.bfloat16
AF = mybir.ActivationFunctionType
ALU = mybir.AluOpType

T = 8
B_FULL = 128
N_CORES = 8
BL = B_FULL // N_CORES  # 16 samples per core
EPS = 1e-5


# --------------------------------------------------------------------------
# Custom DVE ops (fused LIF pieces)
# --------------------------------------------------------------------------

def _register_op(name, spec, ref):
    for op in _dops.OPS:
        if op.name == name:
            return op
    spec = Spec(body=spec.body, accum=spec.accum, accum_init=spec.accum_init,
                reference=ref)
    shas = {}
    for ver in ("v3", "v4"):
        s = DveOpSpec(name=name, opcode=0, uops=lower(spec, ver=ver),
                      rd1_en=_has_src1(spec))
        shas[ver] = s.sha(ver)
    op = _dops.DveOp(name, spec, subdim=False, uops_sha=shas)
    _dops.OPS.append(op)
    _dops.CUSTOM_DVE_SPECS[name] = spec
    _dops._SUB_OPCODE_FOR_NAME[name] = max(_dops._SUB_OPCODE_FOR_NAME.values()) + 1
    return op


# A_t = (A_{t-1} if A_{t-1} < theta_{t-1} else 0) + y*scale + bias
LIF_CHARGE = _register_op(
    "LIF_CHARGE_SNN",
    Spec(body=select(Src0 >= C2, Zero, Src0) + Src1 * C0 + C1),
    lambda in0, in1, s0, s1, imm2: np.where(in0 >= imm2, 0.0, in0) + in1 * s0 + s1,
)

# spike counts over horizontal pairs: (a>=th) + (b>=th)   (values 0/1/2)
SPIKE_HPOOL = _register_op(
    "SPIKE_HPOOL_SNN",
    Spec(body=(Src0 >= C0) + (Src1 >= C0)),
    lambda in0, in1, s0, s1, imm2: (in0 >= s0).astype(np.float32)
    + (in1 >= s0).astype(np.float32),
)

# pooled = (hpA + hpB) * 0.25
VPOOL_SCALE = _register_op(
    "VPOOL_SCALE_SNN",
    Spec(body=(Src0 + Src1) * C1),
    lambda in0, in1, s0, s1, imm2: (in0 + in1) * s1,
)

# square + row-sum (for BN sum-of-squares on the vector engine)
SQ_ACC = _register_op(
    "SQ_ACC_SNN",
    Spec(body=sq(Src0), accum=AluOp.ADD),
    lambda in0, s0, s1, imm2: in0 * in0,
)

# plain spike: (a >= th)
SPIKE_GE = _register_op(
    "SPIKE_GE_SNN",
    Spec(body=(Src0 >= C0) + Zero),
    lambda in0, s0, s1, imm2: (in0 >= s0).astype(np.float32),
)


# --------------------------------------------------------------------------
# Kernel build
# --------------------------------------------------------------------------

def _stats_to_scale_bias(nc, pool, tot, g_dr, b_dr, n_count, nch, out_scale, out_bias):
    """tot: [nch,2] SBUF (sum, sumsq). Writes out_scale/out_bias [nch,8]:
    scale[:,t] = gamma*rstd*2^t ; bias[:,t] = (beta - mu*gamma*rstd)*2^t."""
    mu = pool.tile([nch, 1], F32)
    nc.vector.tensor_scalar_mul(mu[:], tot[:, 0:1], 1.0 / n_count)
    e2 = pool.tile([nch, 1], F32)
    nc.vector.tensor_scalar_mul(e2[:], tot[:, 1:2], 1.0 / n_count)
    var = pool.tile([nch, 1], F32)
    nc.vector.tensor_tensor(var[:], mu[:], mu[:], ALU.mult)
    nc.vector.tensor_tensor(var[:], e2[:], var[:], ALU.subtract)
    nc.vector.tensor_scalar_add(var[:], var[:], float(EPS))
    std = pool.tile([nch, 1], F32)
    nc.scalar.activation(std[:], var[:], AF.Sqrt, bias=0.0, scale=1.0)
    rstd = pool.tile([nch, 1], F32)
    nc.vector.reciprocal(rstd[:], std[:])
    gam = pool.tile([nch, 1], F32)
    nc.sync.dma_start(gam[:], g_dr.ap()[:, None])
    bet = pool.tile([nch, 1], F32)
    nc.sync.dma_start(bet[:], b_dr.ap()[:, None])
    gr = pool.tile([nch, 1], F32)
    nc.vector.tensor_tensor(gr[:], gam[:], rstd[:], ALU.mult)
    bb = pool.tile([nch, 1], F32)  # beta - mu*gr
    nc.vector.tensor_tensor(bb[:], mu[:], gr[:], ALU.mult)
    nc.vector.tensor_tensor(bb[:], bet[:], bb[:], ALU.subtract)
    for t in range(T):
        nc.vector.tensor_scalar_mul(out_scale[:nch, t : t + 1], gr[:], float(2.0**t))
        nc.vector.tensor_scalar_mul(out_bias[:nch, t : t + 1], bb[:], float(2.0**t))


def _allreduce(nc, dram_pool, sb_pool, src_ap, shape):
    """AllReduce-add src_ap ([P,F] SBUF) across all 8 cores; returns SBUF tile."""
    bin_ = dram_pool.tile(list(shape), F32)
    bout = dram_pool.tile(list(shape), F32)
    nc.gpsimd.dma_start(bin_[:], src_ap)
    nc.gpsimd.collective_compute(
        "AllReduce", ALU.add,
        replica_groups=[list(range(N_CORES))],
        ins=[bin_.opt()], outs=[bout.opt()],
    )
    res = sb_pool.tile(list(shape), F32)
    nc.gpsimd.dma_start(res[:], bout[:])
    return res


def build(nc):
    # ---- DRAM I/O -------------------------------------------------------
    x_seq = nc.dram_tensor("x_seq", [T, BL, 3, 32, 32], F32, kind="ExternalInput")
    w1_dr = nc.dram_tensor("conv1_w", [64, 3, 3, 3], F32, kind="ExternalInput")
    g1_dr = nc.dram_tensor("bn1_g", [64], F32, kind="ExternalInput")
    b1_dr = nc.dram_tensor("bn1_b", [64], F32, kind="ExternalInput")
    w2_dr = nc.dram_tensor("conv2_w", [128, 64, 3, 3], F32, kind="ExternalInput")
    g2_dr = nc.dram_tensor("bn2_g", [128], F32, kind="ExternalInput")
    b2_dr = nc.dram_tensor("bn2_b", [128], F32, kind="ExternalInput")
    fc1_dr = nc.dram_tensor("fc1_w", [256, 8192], F32, kind="ExternalInput")
    fc2_dr = nc.dram_tensor("fc2_w", [10, 256], F32, kind="ExternalInput")
    fc2b_dr = nc.dram_tensor("fc2_b", [10], F32, kind="ExternalInput")
    out_dr = nc.dram_tensor("out", [T, BL, 10], F32, kind="ExternalOutput")

    with tile.TileContext(nc) as tc:
        import contextlib
        with contextlib.ExitStack() as ctx:
            dram = ctx.enter_context(tc.tile_pool(name="dram", bufs=1, space="DRAM"))
            persist = ctx.enter_context(tc.tile_pool(name="persist", bufs=1))

            # internal DRAM for layer outputs (pre-BN conv results, fp32)
            y1_dram = dram.tile([8, 16, 128, 512], F32)   # [pair, seg=(t,hh), part, col]
            y2_dram = dram.tile([T, BL, 128, 256], F32)   # [t, b, ch, hw]

            # persistent small tensors
            scale1 = persist.tile([128, T], F32)
            bias1 = persist.tile([128, T], F32)
            scale2 = persist.tile([128, T], F32)
            bias2 = persist.tile([128, T], F32)
            s1buf = persist.tile([128, 64], F32)
            s2buf = persist.tile([128, 64], F32)
            s1buf2 = persist.tile([128, 32], F32)
            s2buf2 = persist.tile([128, 32], F32)

            # conv1 weights: row r = dx*9+dy*3+ci, duplicated on 4 strips
            w1f = persist.tile([27, 64], F32)
            for dy in range(3):
                for dx in range(3):
                    r0 = dy * 9 + dx * 3
                    nc.sync.dma_start(
                        w1f[r0 : r0 + 3, :],
                        w1_dr.ap()[:, :, dy, dx].rearrange("c ci -> ci c"),
                    )
            w1ab = persist.tile([128, 64], BF16)  # rows 0-26 Whi, 27-53 Wlo, 64-90 Whi
            w1tmp = persist.tile([27, 64], BF16)
            nc.vector.tensor_copy(w1ab[0:27], w1f[:])
            nc.vector.tensor_tensor(w1tmp[:], w1f[:], w1ab[0:27], ALU.subtract)
            nc.sync.dma_start(w1ab[27:54], w1tmp[:])
            nc.sync.dma_start(w1ab[54:81], w1ab[0:27])

            # conv2 weights per shift-group g=(dy,dx): rows 0-63 = W_hi (64ci),
            # rows 64-127 = W_lo -> one K=128 matmul per g against duplicated x
            w2f = persist.tile([64, 9, 128], F32)
            nc.sync.dma_start(w2f[:], w2_dr.ap().rearrange("c ci dy dx -> ci (dy dx) c"))
            w2cat = persist.tile([128, 9, 128], BF16)
            w2tmp = persist.tile([64, 9, 128], BF16)
            nc.vector.tensor_copy(w2cat[0:64], w2f[:])
            nc.vector.tensor_tensor(w2tmp[:], w2f[:], w2cat[0:64], ALU.subtract)
            nc.sync.dma_start(w2cat[64:128], w2tmp[:])

            # =============== STAGE A: conv1 + stats + store ===============
            # Base padded planes staged once in DRAM (1.8 MB); the im2col tile
            # rows are overlapping shifted windows of those planes, loaded with
            # one multi-dim strided-AP DMA per 27-row block (contiguous 74KB
            # per-partition runs). Quarters (32 frames) double-buffer so loads
            # overlap matmuls.
            GUARD = 64
            PLANE = 128 * 1156  # full (t b) stream of padded 34x34 frames
            PSTR = GUARD + PLANE + GUARD
            xbase_hi = dram.tile([3, PSTR], BF16, name="xbh")
            xbase_lo = dram.tile([3, PSTR], BF16, name="xbl")
            with tc.tile_pool(name="xstage", bufs=1) as xst:
                # x staged with (t b) on partitions: padded frames are
                # contiguous per partition -> 2.3KB DMA runs to DRAM.
                xpadF = xst.tile([128, 3, 34, 34], F32)
                nc.vector.memset(xpadF[:], 0.0)
                xraw = xst.tile([128, 3, 32, 32], F32)
                nc.sync.dma_start(
                    xraw.rearrange("p c h w -> p (c h w)"),
                    x_seq.ap().rearrange("t b c h w -> (t b) (c h w)"),
                )
                nc.scalar.copy(xpadF[:, :, 1:33, 1:33], xraw[:])
                xpad_flat = xpadF.rearrange("p c h w -> p (c h w)")
                x_hiF = xst.tile([128, 3, 34, 34], BF16)
                x_loF = xst.tile([128, 3, 34, 34], BF16)
                xhi_flat = x_hiF.rearrange("p c h w -> p (c h w)")
                xlo_flat = x_loF.rearrange("p c h w -> p (c h w)")
                nc.vector.tensor_copy(xhi_flat[:], xpad_flat[:])
                nc.vector.tensor_tensor(xlo_flat[:], xpad_flat[:],
                                        xhi_flat[:], ALU.subtract)
                for ci in range(3):
                    nc.sync.dma_start(
                        xbase_hi[ci, GUARD : GUARD + PLANE]
                            .rearrange("(tb f) -> tb f", f=1156),
                        x_hiF[:, ci].rearrange("p h w -> p (h w)"),
                    )
                    nc.scalar.dma_start(
                        xbase_lo[ci, GUARD : GUARD + PLANE]
                            .rearrange("(tb f) -> tb f", f=1156),
                        x_loF[:, ci].rearrange("p h w -> p (h w)"),
                    )

            # im2col row r = dy*9+dx*3+ci reads plane ci at shift
            # (dy-1)*34+(dx-1): a single 4-d AP [dy:34, dx:1, ci:PSTR,
            # cols:1] based at GUARD-35 covers all 27 rows.
            QCOLS = 32 * 1156  # 36992 cols per quarter (32 frames)

            def imc_src(base_tile, q, dy, dx):
                a = base_tile[:]
                off = (a.offset + GUARD + (dy - 1) * 34 + (dx - 1)
                       + q * QCOLS)
                return bass.AP(a.tensor, off, [[PSTR, 3], [1, QCOLS]])

            with tc.tile_pool(name="psumA", bufs=4, space="PSUM") as psum, \
                 tc.tile_pool(name="ysb", bufs=3) as ysb_pool, \
                 tc.tile_pool(name="sq", bufs=2) as sq_pool, \
                 tc.tile_pool(name="imc", bufs=2) as imc_pool:
                for q in range(4):  # quarter = t in {2q, 2q+1}
                    imc = imc_pool.tile([128, QCOLS], BF16, tag="imc", name="imc")
                    imc_v = imc.rearrange("p (tb h w) -> p tb h w", h=34, w=34)
                    for g in range(9):
                        dy, dx = g // 3, g % 3
                        r0 = 3 * g
                        nc.sync.dma_start(imc[r0 : r0 + 3, :],
                                          imc_src(xbase_hi, q, dy, dx))
                        nc.scalar.dma_start(imc[27 + r0 : 27 + r0 + 3, :],
                                            imc_src(xbase_hi, q, dy, dx))
                        nc.gpsimd.dma_start(imc[54 + r0 : 54 + r0 + 3, :],
                                            imc_src(xbase_lo, q, dy, dx))
                    for tl in range(2):
                        t = 2 * q + tl
                        for p in range(8):
                            idx = t * 8 + p
                            ps = psum.tile([128, 1024], F32, tag="ps", name="ps")
                            for hh in range(2):
                                h0 = hh * 16
                                for bhalf in range(2):
                                    b = 2 * p + bhalf
                                    fr = tl * 16 + b
                                    rhs = imc_v[:, fr, h0 + 1 : h0 + 17, 1:33]
                                    nc.tensor.matmul(
                                        ps[64 * bhalf : 64 * bhalf + 64,
                                           512 * hh : 512 * hh + 512],
                                        w1ab[0:81, :], rhs[0:81],
                                        start=True, stop=True,
                                        tile_position=(0, 64 * bhalf),
                                    )
                            y_sb = ysb_pool.tile([128, 1024], F32)
                            nc.scalar.activation(y_sb[:], ps[:], AF.Identity,
                                                 bias=0.0, scale=1.0,
                                                 accum_out=s1buf[:, idx : idx + 1])
                            sq_t = sq_pool.tile([128, 1024], F32, name="sqsc")
                            nc.vector._custom_dve(
                                SQ_ACC, out=sq_t[:], in0=ps[:],
                                accum_out=s2buf[:, idx : idx + 1])
                            nc.gpsimd.dma_start(
                                y1_dram[p, 2 * t : 2 * t + 2].rearrange(
                                    "s part c -> part s c"),
                                y_sb.rearrange("p (s c) -> p s c", s=2))

            # =============== BN1 stats + allreduce ===============
            sums1 = persist.tile([128, 2], F32)
            nc.vector.tensor_reduce(sums1[:, 0:1], s1buf[:], mybir.AxisListType.X, ALU.add)
            nc.vector.tensor_reduce(sums1[:, 1:2], s2buf[:], mybir.AxisListType.X, ALU.add)
            g1 = _allreduce(nc, dram, persist, sums1[:], (128, 2))
            par1 = persist.tile([64, 2], F32)
            nc.sync.dma_start(par1[:], g1[64:128, :])
            tot1 = persist.tile([64, 2], F32)
            nc.vector.tensor_tensor(tot1[:], g1[0:64, :], par1[:], ALU.add)
            _stats_to_scale_bias(nc, persist, tot1, g1_dr, b1_dr,
                                 float(T * B_FULL * 32 * 32), 64, scale1, bias1)
            nc.sync.dma_start(scale1[64:128, :], scale1[0:64, :])
            nc.sync.dma_start(bias1[64:128, :], bias1[0:64, :])

            # fc weights + pooled2 (allocated after stage A frees its SBUF)
            fcpool = ctx.enter_context(tc.tile_pool(name="fcpool", bufs=1))
            fc1w_hi = fcpool.tile([128, 256, 64], BF16)
            fc1w_lo = fcpool.tile([128, 256, 64], BF16)
            with tc.tile_pool(name="fcstage", bufs=1) as fst:
                for m in range(2):
                    stg = fst.tile([128, 128, 64], F32, tag="fcs", name="fcs")
                    nc.sync.dma_start(
                        stg[:],
                        fc1_dr.ap()[m * 128 : (m + 1) * 128]
                            .rearrange("o (r k) -> r o k", r=128),
                    )
                    hi_m = fc1w_hi[:, m * 128 : (m + 1) * 128, :]
                    nc.vector.tensor_copy(hi_m, stg[:])
                    nc.vector.tensor_tensor(
                        fc1w_lo[:, m * 128 : (m + 1) * 128, :],
                        stg[:], hi_m, ALU.subtract)
            fc2w = fcpool.tile([128, 2, 10], F32)  # [r, m, o] ; i = m*128+r
            for m in range(2):
                nc.sync.dma_start(
                    fc2w[:, m, :],
                    fc2_dr.ap()[:, m * 128 : (m + 1) * 128].rearrange("o r -> r o"),
                )
            fc2b = fcpool.tile([10, 1], F32)
            nc.sync.dma_start(fc2b[:], fc2b_dr.ap()[:, None])
            pooled2 = fcpool.tile([128, 8192], BF16)  # [(c), (t b hw)]

            # =============== STAGE B: LIF1 + pool ===============
            with tc.tile_pool(name="pooled1_pool", bufs=1) as pp1:
                pooled1 = pp1.tile([128, T, 8, 18, 18], BF16)
                nc.vector.memset(pooled1[:], 0.0)
                with tc.tile_pool(name="stageB", bufs=6) as pB, \
                     tc.tile_pool(name="stateB", bufs=1) as stB:
                    for p in range(8):
                        st = [stB.tile([128, 1024], F32, tag=f"st{i}", name=f"stB{i}") for i in range(2)]
                        nc.vector.memset(st[0][:], 0.0)
                        for t in range(T):
                            yc = pB.tile([128, 2, 512], F32, tag="yc", name="ycB")
                            nc.sync.dma_start(
                                yc[:],
                                y1_dram[p, 2 * t : 2 * t + 2].rearrange(
                                    "s part c -> part s c"),
                            )
                            a_new, a_old = st[(t + 1) % 2], st[t % 2]
                            nc.vector._custom_dve(
                                LIF_CHARGE, out=a_new[:], in0=a_old[:],
                                in1=yc.rearrange("p s c -> p (s c)"),
                                s0=scale1[:, t : t + 1], s1=bias1[:, t : t + 1],
                                imm2=float(2.0**t),
                            )
                            av = a_new.rearrange("p (h w) -> p h w", h=32)
                            hp = pB.tile([128, 32, 16], F32, tag="hp")
                            nc.vector._custom_dve(
                                SPIKE_HPOOL, out=hp[:],
                                in0=av[:, :, 0:32:2], in1=av[:, :, 1:32:2],
                                s0=float(2.0 ** (t + 1)),
                            )
                            nc.vector._custom_dve(
                                VPOOL_SCALE,
                                out=pooled1[:, t, p, 1:17, 1:17],
                                in0=hp[:, 0:32:2, :], in1=hp[:, 1:32:2, :],
                                s1=0.25,
                            )

                # =============== STAGE C: conv2 + stats + store ===============
                # pooled1 duplicated across partition halves per t (samples
                # interleaved) so each shift-group is one K=128 matmul against
                # [W_hi; W_lo] stacked rows.
                with tc.tile_pool(name="ysb2", bufs=3) as ysb2_pool, \
                     tc.tile_pool(name="psumC", bufs=4, space="PSUM") as psum, \
                     tc.tile_pool(name="sq2", bufs=2) as sq2_pool, \
                     tc.tile_pool(name="dup", bufs=2) as dup_pool:
                    for t in range(T):
                        dup = dup_pool.tile([128, 16, 18, 18], BF16,
                                            tag="dup", name="dup")
                        nc.sync.dma_start(dup[0:64, 0:16:2], pooled1[0:64, t])
                        nc.scalar.dma_start(dup[0:64, 1:16:2], pooled1[64:128, t])
                        nc.gpsimd.dma_start(dup[64:128, 0:16:2], pooled1[0:64, t])
                        nc.sync.dma_start(dup[64:128, 1:16:2], pooled1[64:128, t])
                        for sp2 in range(4):  # 4 samples per psum tile
                            cidx = t * 4 + sp2
                            ps = psum.tile([128, 1024], F32, tag="ps", name="psc")
                            for half2 in range(2):
                                s0 = 4 * sp2 + 2 * half2
                                for g in range(9):
                                    dy, dx = g // 3, g % 3
                                    nc.tensor.matmul(
                                        ps[:, 512 * half2 : 512 * half2 + 512],
                                        w2cat[:, g, :],
                                        dup[:, s0 : s0 + 2,
                                            dy : dy + 16, dx : dx + 16],
                                        start=(g == 0), stop=(g == 8),
                                    )
                            y_sb = ysb2_pool.tile([128, 1024], F32)
                            nc.scalar.activation(y_sb[:], ps[:], AF.Identity,
                                                 bias=0.0, scale=1.0,
                                                 accum_out=s1buf2[:, cidx : cidx + 1])
                            sq_t = sq2_pool.tile([128, 1024], F32, name="sqsc2")
                            nc.scalar.activation(sq_t[:], ps[:], AF.Square,
                                                 bias=0.0, scale=1.0,
                                                 accum_out=s2buf2[:, cidx : cidx + 1])
                            nc.sync.dma_start(
                                y2_dram[t, 4 * sp2 : 4 * sp2 + 4].rearrange(
                                    "b p c -> p b c"),
                                y_sb.rearrange("p (b c) -> p b c", b=4),
                            )

            # =============== BN2 stats + allreduce ===============
            sums2 = persist.tile([128, 2], F32)
            nc.vector.tensor_reduce(sums2[:, 0:1], s1buf2[:], mybir.AxisListType.X, ALU.add)
            nc.vector.tensor_reduce(sums2[:, 1:2], s2buf2[:], mybir.AxisListType.X, ALU.add)
            g2 = _allreduce(nc, dram, persist, sums2[:], (128, 2))
            _stats_to_scale_bias(nc, persist, g2, g2_dr, b2_dr,
                                 float(T * B_FULL * 16 * 16), 128, scale2, bias2)

            # =============== STAGE D: LIF2 + pool ===============
            with tc.tile_pool(name="stageD", bufs=3) as pD, \
                 tc.tile_pool(name="stateD", bufs=1) as stD:
                for bp in range(8):  # b-pairs
                    b0 = 2 * bp
                    yc = pD.tile([128, T, 2, 256], F32)
                    for t in range(T):
                        nc.sync.dma_start(
                            yc[:, t],
                            y2_dram[t, b0 : b0 + 2].rearrange("b p c -> p b c"),
                        )
                    ycv = yc.rearrange("p t b c -> p t (b c)")
                    st = [stD.tile([128, 512], F32, tag=f"std{i}", name=f"stD{i}") for i in range(2)]
                    nc.vector.memset(st[0][:], 0.0)
                    for t in range(T):
                        a_new, a_old = st[(t + 1) % 2], st[t % 2]
                        nc.vector._custom_dve(
                            LIF_CHARGE, out=a_new[:], in0=a_old[:],
                            in1=ycv[:, t, :],
                            s0=scale2[:, t : t + 1], s1=bias2[:, t : t + 1],
                            imm2=float(2.0**t),
                        )
                        av = a_new.rearrange("p (bh w) -> p bh w", w=16)
                        hp = pD.tile([128, 32, 8], F32, tag="hp2")
                        nc.vector._custom_dve(
                            SPIKE_HPOOL, out=hp[:],
                            in0=av[:, :, 0:16:2], in1=av[:, :, 1:16:2],
                            s0=float(2.0 ** (t + 1)),
                        )
                        pout = pooled2[:, (t * 16 + b0) * 64 : (t * 16 + b0 + 2) * 64]
                        nc.vector._custom_dve(
                            VPOOL_SCALE,
                            out=pout.rearrange("p (bh w) -> p bh w", w=8),
                            in0=hp[:, 0:32:2, :], in1=hp[:, 1:32:2, :],
                            s1=0.25,
                        )

            # =============== STAGE E: fc1 + LIF + fc2 ===============
            p2v = pooled2.rearrange("p (tb k) -> p tb k", k=64)
            with tc.tile_pool(name="stageE", bufs=1) as pE, \
                 tc.tile_pool(name="psumE", bufs=2, space="PSUM") as psE:
                s_sb = pE.tile([128, 2, T, BL], F32)
                for m in range(2):
                    psf = psE.tile([128, 128], F32, tag="psf")
                    nmm = 0
                    for wt in (fc1w_hi, fc1w_lo):
                        for k in range(64):
                            nc.tensor.matmul(
                                psf[:], wt[:, m * 128 : (m + 1) * 128, k],
                                p2v[:, :, k],
                                start=(nmm == 0), stop=(nmm == 127),
                            )
                            nmm += 1
                    stf = [pE.tile([128, BL], F32, tag=f"stf{i}", name=f"stf{i}") for i in range(2)]
                    nc.vector.memset(stf[0][:], 0.0)
                    for t in range(T):
                        a_new, a_old = stf[(t + 1) % 2], stf[t % 2]
                        nc.vector._custom_dve(
                            LIF_CHARGE, out=a_new[:], in0=a_old[:],
                            in1=psf[:, t * BL : (t + 1) * BL],
                            s0=float(2.0**t), s1=0.0, imm2=float(2.0**t),
                        )
                        nc.vector._custom_dve(
                            SPIKE_GE, out=s_sb[:, m, t, :], in0=a_new[:],
                            s0=float(2.0 ** (t + 1)),
                        )
                pso = psE.tile([10, 128], F32, tag="pso")
                sv = s_sb.rearrange("p m t b -> p m (t b)")
                nc.tensor.matmul(pso[:], fc2w[:, 0, :], sv[:, 0, :],
                                 start=True, stop=False)
                nc.tensor.matmul(pso[:], fc2w[:, 1, :], sv[:, 1, :],
                                 start=False, stop=True)
                out_sb = pE.tile([10, 128], F32)
                nc.scalar.activation(out_sb[:], pso[:], AF.Identity,
                                     bias=fc2b[:, 0:1], scale=1.0)
                nc.sync.dma_start(out_dr.ap().rearrange("t b o -> o (t b)"), out_sb[:])

    return nc


_CACHED = None


def _get_compiled():
    global _CACHED
    if _CACHED is None:
        nc = bacc.Bacc("TRN2", target_bir_lowering=False, debug=False,
                       num_devices=N_CORES)
        build(nc)
        nc.compile()
        _CACHED = nc
    return _CACHED


def kernel(**inputs) -> np.ndarray:
    nc = _get_compiled()
    np_in = {k: np.ascontiguousarray(np.asarray(v, dtype=np.float32))
             for k, v in inputs.items()}
    in_maps = []
    for i in range(N_CORES):
        m = dict(np_in)
        m["x_seq"] = np.ascontiguousarray(
            np_in["x_seq"][:, i * BL : (i + 1) * BL])
        in_maps.append(m)
    res = bass_utils.run_bass_kernel_spmd(nc, in_maps, core_ids=list(range(N_CORES)))
    return np.concatenate([res.results[i]["out"] for i in range(N_CORES)], axis=1)


if __name__ == "__main__":
    nc = _get_compiled()
    print("compiled OK")

